# revision 20
# baseline (speedup 1.0000x reference)
"""Trainium2 Bass kernel for nn_Conv2dModulation.

Math (per sample s):
    w0 = weight * c,  c = (cin*3*3)^-0.5
    w1[o,i,kh,kw] = w0[o,i,kh,kw] * y[s,i]
    d[o] = rsqrt(sum_{i,kh,kw} w1^2 + eps)
    out[s] = conv2d_SAME(X[s], w1 * d)

All weight math (modulate + demodulate) is done on the HOST in fp32 and
folded into a single bf16 stationary tensor wm[(s,i),(t,o)].  The device
then runs a pure 9-tap conv: nothing gates the PE except the small wm
DMA and the first X rows, and PSUM evacuation is a plain dtype-cast
copy (no scale).  (A previous revision computed the demod chain on
device; its matmuls head-of-line blocked the PE queue behind a DVE
reduce + ACT sqrt table load, costing ~7us of PE idle at startup.)

Device strategy (per core, 2 samples):
  - PE warmup: ~3us of dummy 4-quadrant matmuls during the input-DMA
    wait burn the HAM cold window (PE at 1.2 GHz for its first ~3.4us
    of activity) so the real conv starts at 2.4 GHz.
  - X zero-padded on host to [H+2, W+2] bf16.  The weights + X rows
    0-3 arrive as ONE DMA packet per partition (host-packed "head"
    tensor) since startup is packet-latency-bound; the rest of chunk 0
    in need-ordered slices; later chunks one DMA each (xpool
    triple-buffered).  Input DMAs ride the SP DGE ring; output DMAs
    ride the ACT ring so a blocked chunk-prefetch push can never delay
    them (ostage->PSUM backpressure stalls the PE otherwise).
  - Conv = 9 shifted matmuls (taps) accumulating into PSUM.  All four
    64x64 PE-array quadrants run concurrently: rows = sample (s),
    cols = output-row pair (q), via tile_position=(s*64, q*64).  The
    (s,q) matmul writes PSUM bank_s partitions [q*64:(q+1)*64].
    Steady-state cadence is the back-to-back floor (512 cyc @ 2.4 GHz
    + ~2.5 ns issue) per quadrant.
  - PSUM (fp32) -> SBUF bf16: DVE tensor_copy (s=0) / ACT
    activation-Copy (s=1), partition remap (q,o)->(s,o); one batched
    DMA per 2 row-groups (8 output rows) back to HBM in bf16 (host
    converts to fp32).  The final 4 rows run as two 2-row PSUM groups
    (N=256) flushed immediately from the idle SP ring, halving the
    tail evacuation.
"""

import numpy as np
import ml_dtypes

import concourse.bass as bass
import concourse.tile as tile
from concourse import bacc, mybir
from concourse.bass_utils import run_bass_kernel_spmd

F32 = mybir.dt.float32
BF16 = mybir.dt.bfloat16
NPBF16 = ml_dtypes.bfloat16

B, C, H, W, KS = 16, 64, 256, 256, 3
NCORES = 8
SPC = B // NCORES          # samples per core = 2
WP = W + 2                 # padded row width
HP = H + 2                 # padded column height
EPS = 1e-8
CKAIMING = float((C * KS * KS) ** -0.5)
NW = KS * KS * C           # 576

R = 32                     # output rows per chunk
NCHUNK = H // R

XT_BUFS = 3


HEAD_ROWS = 6              # X rows packed with the weights in one DMA


def build_program(nc):
    Xl = nc.dram_tensor("Xl", [SPC * C, HP, WP], BF16, kind="ExternalInput")
    # head packs, per partition (s,i): wm[(s,i),(t,o)] (the modulated+
    # demodulated weights, NW cols) followed by X rows 0..HEAD_ROWS-1.
    # One contiguous run per partition = one DMA packet, so the data
    # gating the first matmul rides the minimum number of ~80ns-latency
    # DMA packets.
    head = nc.dram_tensor("head", [2 * C, NW + HEAD_ROWS * WP], BF16,
                          kind="ExternalInput")
    out = nc.dram_tensor("out", [SPC * C, H, W], BF16, kind="ExternalOutput")

    with tile.TileContext(nc) as tc:
        with (
            tc.tile_pool(name="wpool", bufs=1) as wpool,
            tc.tile_pool(name="xpool", bufs=XT_BUFS) as xpool,
            tc.tile_pool(name="opool", bufs=8) as opool,
            tc.tile_pool(name="psA", bufs=4, space="PSUM") as psA,
            tc.tile_pool(name="psB", bufs=4, space="PSUM") as psB,
        ):
            # PE warmup: the HAM clock gate holds the PE at 1.2 GHz for
            # its first ~3.4us of activity.  Burn that cold window on
            # dummy matmuls (all 4 quadrants, for full HAM activity
            # credit) while the input DMAs are still in flight, so the
            # real conv starts at 2.4 GHz.  The memset feeding them
            # runs on the otherwise-idle DVE.  7 rounds x ~427ns end
            # just before the first X rows land; the short idle after
            # is well under the ~3.4us re-throttle window.
            warm = wpool.tile([2 * C, NW], BF16)
            nc.vector.memset(warm[:, :], 0.0)
            warmA = psA.tile([2 * C, 512], F32, name="warmA", tag="p00")
            warmB = psB.tile([2 * C, 512], F32, name="warmB", tag="p10")
            for _ in range(7):
                for rh in range(2):
                    wps = (warmA, warmB)[rh]
                    for ch in range(2):
                        nc.tensor.matmul(
                            wps[ch * C:(ch + 1) * C, :],
                            warm[rh * C:(rh + 1) * C, 0:C],
                            warm[rh * C:(rh + 1) * C, C:C + 512],
                            start=True, stop=True,
                            tile_position=(rh * C, ch * C),
                            skip_group_check=True)

            # one SBUF tile holds wm + all of chunk 0, so matmul APs can
            # span slice boundaries freely.  The first-matmul gate
            # (wm + rows 0-3) arrives as ONE DMA packet per partition;
            # rows 4-5 (first needed by tap 3, ~0.65us into the conv)
            # follow as a second packet.
            bigx = wpool.tile([2 * C, NW + (R + 2) * WP], BF16)
            nc.sync.dma_start(bigx[:, 0:NW + 4 * WP],
                              head.ap()[:, 0:NW + 4 * WP])
            nc.sync.dma_start(bigx[:, NW + 4 * WP:NW + HEAD_ROWS * WP],
                              head.ap()[:, NW + 4 * WP:])
            wmt = bigx[:, 0:NW]
            xt0v = bigx[:, NW:].rearrange("p (r w) -> p r w", w=WP)
            for lo, hi in ((HEAD_ROWS, 10), (10, 18), (18, R + 2)):
                nc.sync.dma_start(xt0v[:, lo:hi, :], Xl.ap()[:, lo:hi, :])

            # ---- conv main loop ----
            for ci in range(NCHUNK):
                r0 = ci * R
                if ci == 0:
                    xt3 = xt0v
                else:
                    xt = xpool.tile([2 * C, (R + 2) * WP], BF16)
                    xt3 = xt[:, :].rearrange("p (r w) -> p r w", w=WP)
                    nc.sync.dma_start(xt3[:, :, :],
                                      Xl.ap()[:, r0:r0 + R + 2, :])

                for rbp in range(R // 8):          # pairs of row-groups
                    last_pair = (ci == NCHUNK - 1 and rbp == R // 8 - 1)
                    ostage = opool.tile([2 * C, 2 * 1024], BF16)
                    if last_pair:
                        # final 8 output rows: four 2-row PSUM groups
                        # (N=256, q = single row), each evacuated and
                        # flushed as soon as its taps finish, so the
                        # out-DMA queue drains DURING the last matmuls
                        # instead of serially after them.  Pushes from
                        # the by-now-idle SP engine keep ACT's
                        # evacuations back-to-back.
                        for hh in range(4):
                            psh = [
                                psA.tile([2 * C, 512], F32,
                                         name=f"psh0_{hh}", tag="p00"),
                                psB.tile([2 * C, 512], F32,
                                         name=f"psh1_{hh}", tag="p10"),
                            ]
                            for t in range(KS * KS):
                                dh, dw = t // KS - 1, t % KS - 1
                                for s in range(SPC):
                                    lhsT = wmt[s * C:(s + 1) * C,
                                               t * C:(t + 1) * C]
                                    for q in range(2):
                                        lr = (rbp * 8 + hh * 2 + q
                                              + dh + 1)
                                        co = dw + 1
                                        rhs = xt3[s * C:(s + 1) * C,
                                                  lr:lr + 1,
                                                  co:co + W]
                                        nc.tensor.matmul(
                                            psh[s][q * C:(q + 1) * C,
                                                   0:256],
                                            lhsT,
                                            rhs,
                                            start=(t == 0),
                                            stop=(t == KS * KS - 1),
                                            tile_position=(s * C,
                                                           q * C),
                                            skip_group_check=True,
                                        )
                            base = hh * 512
                            for q in range(2):
                                nc.vector.tensor_copy(
                                    ostage[0:C,
                                           base + q * 256:
                                           base + (q + 1) * 256],
                                    psh[0][q * C:(q + 1) * C, 0:256])
                                nc.scalar.activation(
                                    ostage[C:2 * C,
                                           base + q * 256:
                                           base + (q + 1) * 256],
                                    psh[1][q * C:(q + 1) * C, 0:256],
                                    mybir.ActivationFunctionType.Copy)
                            rr = r0 + rbp * 8 + hh * 2
                            nc.sync.dma_start(
                                out.ap()[:, rr:rr + 2, :].rearrange(
                                    "so t w -> so (t w)"),
                                ostage[:, base:base + 512])
                        continue
                    for g in range(2):
                        rb = rbp * 2 + g
                        ps = [
                            psA.tile([2 * C, 512], F32,
                                     name=f"ps0_{ci}_{rb}", tag="p00"),
                            psB.tile([2 * C, 512], F32,
                                     name=f"ps1_{ci}_{rb}", tag="p10"),
                        ]
                        for t in range(KS * KS):
                            dh, dw = t // KS - 1, t % KS - 1
                            for s in range(SPC):
                                lhsT = wmt[s * C:(s + 1) * C,
                                           t * C:(t + 1) * C]
                                for q in range(2):
                                    lr = rb * 4 + 2 * q + dh + 1
                                    co = dw + 1
                                    rhs = xt3[s * C:(s + 1) * C,
                                              lr:lr + 2, co:co + W]
                                    nc.tensor.matmul(
                                        ps[s][q * C:(q + 1) * C, :],
                                        lhsT,
                                        rhs,
                                        start=(t == 0),
                                        stop=(t == KS * KS - 1),
                                        tile_position=(s * C, q * C),
                                        skip_group_check=True,
                                    )
                        # evacuate PSUM -> SBUF bf16 (plain dtype-cast
                        # copy; demod already folded into weights);
                        # partition remap (q,o) -> (s,o); DVE: s=0,
                        # ACT: s=1 (different banks -> parallel)
                        for q in range(2):
                            nc.vector.tensor_copy(
                                ostage[0:C,
                                       g * 1024 + q * 512:
                                       g * 1024 + (q + 1) * 512],
                                ps[0][q * C:(q + 1) * C, :])
                            nc.scalar.activation(
                                ostage[C:2 * C,
                                       g * 1024 + q * 512:
                                       g * 1024 + (q + 1) * 512],
                                ps[1][q * C:(q + 1) * C, :],
                                mybir.ActivationFunctionType.Copy)
                    rr = r0 + rbp * 8
                    dstap = out.ap()[:, rr:rr + 8, :].rearrange(
                        "so (gq t) w -> so gq (t w)", gq=4)
                    nc.scalar.dma_start(
                        dstap,
                        ostage[:, :].rearrange(
                            "p (gq tw) -> p gq tw", gq=4))

    return nc


_CACHED = {}


def _get_compiled():
    if "nc" not in _CACHED:
        nc = bacc.Bacc("TRN2", debug=False)
        build_program(nc)
        nc.compile()
        _CACHED["nc"] = nc
    return _CACHED["nc"]


def make_in_maps(X, y, weight):
    X = np.ascontiguousarray(X, dtype=np.float32)
    y = np.ascontiguousarray(y, dtype=np.float32)
    weight = np.ascontiguousarray(weight, dtype=np.float32)
    Xp = np.zeros((B, C, HP, WP), dtype=NPBF16)
    Xp[:, :, 1:H + 1, 1:W + 1] = X.astype(NPBF16)
    # host-side modulate + demodulate, folded into one bf16 weight
    w1 = weight[None] * (CKAIMING * y)[:, None, :, None, None]  # [s,o,i,kh,kw]
    d = 1.0 / np.sqrt((w1 * w1).sum(axis=(2, 3, 4), keepdims=True) + EPS)
    wf = (w1 * d).transpose(0, 2, 3, 4, 1)          # [s, i, kh, kw, o]
    wf = np.ascontiguousarray(wf.reshape(B, C, NW))  # [s, i, (t,o)]
    in_maps = []
    for c in range(NCORES):
        xs = np.ascontiguousarray(
            Xp[c * SPC:(c + 1) * SPC].reshape(SPC * C, HP, WP))
        wmc = wf[c * SPC:(c + 1) * SPC].reshape(2 * C, NW).astype(NPBF16)
        headc = np.concatenate(
            [wmc, xs[:, 0:HEAD_ROWS, :].reshape(2 * C, HEAD_ROWS * WP)],
            axis=1)
        in_maps.append({
            "Xl": xs,
            "head": np.ascontiguousarray(headc),
        })
    return in_maps


def kernel(X, y, weight):
    nc = _get_compiled()
    in_maps = make_in_maps(X, y, weight)
    res = run_bass_kernel_spmd(nc, in_maps, core_ids=list(range(NCORES)))
    outs = [res.results[c]["out"].astype(np.float32).reshape(SPC, C, H, W)
            for c in range(NCORES)]
    return np.concatenate(outs, axis=0)
